# revision 1
# baseline (speedup 1.0000x reference)
"""Bass kernel builder for ClassSeparationLossMargin.

loss = mean_ij [ t*(1-cos) + (1-t)*relu(margin - (1-cos)) ],
cos = xn @ xn.T (row-normalized), t = same-class mask.

Per core (on a row-rolled copy of the full input so the same SPMD program
processes "rows 0:N/8" everywhere):
  G = H @ H.T with H = [xn | sqk*O]  (O = one-hot classes) => G = cos + K*t
  relu_sum = weighted sum over scheduled tiles of relu(G + (margin-1))
  corr     = (0.9-K)*A - 2*B,  A = sum_c n_c^2,  B = sum S^2, S = O.T @ xn
  out      = (relu_sum + corr/8 + dve_off) / N^2
Host sums the 8 outputs.

For same-class pairs relu(0.1+cos+K) = 0.1+cos+K exactly (K>0.9), so
sum relu-pass = C_diff + 0.1*A + B + K*A; want A - B + C_diff
=> corr = (0.9-K)*A - 2*B.  K = sqk^2 = 1.265625 (sqk=1.125 exact in bf16).

Triangle: G is symmetric, so local row chunk r only processes col chunks c
with (c - r) mod T in {0..T/2}; weight 1 at the two ends, 2 in the middle.
d = (C-R) mod T is roll-invariant, so across the 8 rolled copies every
unordered pair is counted exactly twice and w1 + 2*w2 = full sum.

Prep is pipelined in groups of TG row-tiles (per-group tiles so the Tile
scheduler overlaps DMA / normalization / transposes / S-matmuls across
groups and with the start of the main loop).
"""

from contextlib import ExitStack

import numpy as np

import concourse.bacc as bacc
import concourse.mybir as mybir
import concourse.tile as tile
from concourse.masks import make_identity

F32 = mybir.dt.float32
BF16 = mybir.dt.bfloat16
I32 = mybir.dt.int32
OP = mybir.AluOpType
AF = mybir.ActivationFunctionType


def _consumer_schedule(N, P, RC, T, cw, triangle):
    """[(segs, weight, engine)] where segs = [(row_chunk, col_off, width)],
    col offsets absolute into hT; engine 'A' = scalar/ACT, 'D' = vector/DVE.
    Triangle w1 chunks (the two span ends, weight 1) are batched 4 rcs per
    consumer op to amortize per-op overhead."""
    ops = []
    if triangle:
        half = T // 2
        for r0 in range(0, RC, 4):
            segs = []
            for r in range(r0, min(r0 + 4, RC)):
                segs.append((r, r * P, P))
                segs.append((r, (r + half) * P, P))
            ops.append((segs, 1, None))
        for r in range(RC):
            span = (half - 1) * P
            off = (r + 1) * P
            while span > 0:
                w = min(cw, span)
                ops.append(([(r, off, w)], 2, None))
                off += w
                span -= w
    else:
        for r in range(RC):
            for c in range(0, N, cw):
                ops.append(([(r, c, cw)], 1, None))
    ta = td = 0.0
    out = []
    for (segs, w, _) in ops:
        fd = sum(s[2] for s in segs)
        ca = (fd + 380) / 1.2     # ACT: op + accum-drain + gap
        cd = (fd + 180) / 0.96    # DVE
        if ta + ca <= td + cd:
            ta += ca
            out.append((segs, w, "A"))
        else:
            td += cd
            out.append((segs, w, "D"))
    return out


def build_nc(N=8192, D=64, C=17, margin=1.1, n_cores=8, cw=1024,
             triangle=True):
    """Inputs: b_t [128, T, D] f32 row-tiled, cm_t [128, T] i32.
    Output: out [1, 1] f32 partial loss."""
    sqk = 1.125
    K = sqk * sqk
    m1 = margin - 1.0            # 0.1
    P = 128
    T = N // P                   # row tiles of the full matrix
    E = D + 1                    # feature cols + norm col
    HD = D + C                   # Gram feature dim (81)
    RC = (N // n_cores) // P     # row chunks this core owns
    TG = min(16, T)              # row tiles per prep group
    NG = T // TG
    GB = TG * P                  # hT columns per group tile

    nc = bacc.Bacc("TRN2", target_bir_lowering=False, num_devices=n_cores)
    b_dram = nc.dram_tensor("b_t", [P, T, D], F32, kind="ExternalInput")
    cm_dram = nc.dram_tensor("cm_t", [P, T], I32, kind="ExternalInput")
    out_dram = nc.dram_tensor("out", [1, 1], F32, kind="ExternalOutput")

    sched = _consumer_schedule(N, P, RC, T, cw, triangle)
    n_a = {1: 0, 2: 0}
    n_d = {1: 0, 2: 0}
    for (segs, w, e) in sched:
        (n_a if e == "A" else n_d)[w] += 1
    # DVE accum quirk: accum = sum_f max(x, -m1) + s2(=0) -> under-counts m1
    # per element (doubled where the column weight is 2).
    dve_off = float(m1 * P * sum(
        sum(s[2] for s in segs) * w
        for (segs, w, e) in sched if e == "D"))
    # main loop only ever reads hT cols [0, (RC + T//2 + 1) * P) in triangle
    # mode -- transposes/copies beyond that are dead work.
    t_used = RC + T // 2 + 1 if triangle else T

    with tile.TileContext(nc) as tc, ExitStack() as top:
        persist = top.enter_context(tc.tile_pool(name="persist", bufs=1))
        prep_es = top.enter_context(ExitStack())
        bpool = prep_es.enter_context(tc.tile_pool(name="bpool", bufs=NG))
        gpool = prep_es.enter_context(tc.tile_pool(name="gpool", bufs=4))
        ps_s = prep_es.enter_context(
            tc.tile_pool(name="ps_s", bufs=1, space="PSUM"))

        # ---- constants (identity first: warm-up matmuls depend on it) ----
        ident = persist.tile([P, P], BF16)
        make_identity(nc, ident[:])
        cm_i = persist.tile([P, T], I32)
        nc.sync.dma_start(cm_i[:], cm_dram[:])
        bias_m1 = persist.tile([P, 1], F32)
        nc.gpsimd.memset(bias_m1[:], m1)
        ones128 = persist.tile([P, 1], F32)
        nc.gpsimd.memset(ones128[:], 1.0)
        ones_c = persist.tile([C, 1], F32)
        nc.gpsimd.memset(ones_c[:], 1.0)
        iota_i = persist.tile([P, TG, C], I32)
        nc.gpsimd.iota(iota_i[:], pattern=[[0, TG], [1, C]], base=0,
                       channel_multiplier=0)
        iotaf = persist.tile([P, TG, C], F32)
        nc.vector.tensor_copy(iotaf[:], iota_i[:])
        iota_b = iotaf[:]

        cm_f = persist.tile([P, T, 1], F32)
        nc.vector.tensor_copy(cm_f[:].squeeze(-1), cm_i[:])

        hT = [persist.tile([HD, GB], BF16, name=f"hT{g}") for g in range(NG)]
        s_ps = ps_s.tile([C, HD], F32)

        # ---- prep: stage-major, few big ops per quarter; all PE work
        # (S-matmuls + transposes) packed dense at the end so HAM warms
        # right before the main loop ----
        with tc.tile_pool(name="ps_a", bufs=2, space="PSUM") as ps_a, \
             tc.tile_pool(name="ps_w", bufs=2, space="PSUM") as ps_w:
            for i in range(24):
                wp = ps_w.tile([P, P], F32, tag="warm", name="wp")
                nc.tensor.matmul(wp[:], ident[:], ident[:],
                                 start=True, stop=True)
            b_gs, s_gs, oh_gs, xno_gs = [], [], [], []
            for g in range(NG):
                b_g = bpool.tile([P, TG, D], F32, tag="b_g", name=f"b_g{g}")
                nc.sync.dma_start(b_g[:], b_dram[:, g * TG:(g + 1) * TG, :])
                b_gs.append(b_g)
            for g in range(NG):
                sq_g = gpool.tile([P, TG, D], F32, tag="sq_g", bufs=2,
                                  name=f"sq{g}")
                nc.scalar.activation(sq_g[:], b_gs[g][:], AF.Square)
                ns_g = gpool.tile([P, TG], F32, tag="ns_g", bufs=NG,
                                  name=f"ns{g}")
                nc.vector.tensor_reduce(ns_g[:], sq_g[:],
                                        axis=mybir.AxisListType.X, op=OP.add)
                nm_g = gpool.tile([P, TG], F32, tag="nm_g", bufs=NG,
                                  name=f"nm{g}")
                nc.scalar.activation(nm_g[:], ns_g[:], AF.Sqrt)
                s_g = gpool.tile([P, TG, 1], F32, tag="s_g", bufs=NG,
                                 name=f"s{g}")
                nc.vector.reciprocal(s_g[:].squeeze(-1), nm_g[:])
                s_gs.append(s_g)
            for g in range(NG):
                oh_g = gpool.tile([P, TG, C], F32, tag="oh_g", bufs=NG,
                                  name=f"oh{g}")
                cm_b = cm_f[:, g * TG:(g + 1) * TG, :].to_broadcast(
                    [P, TG, C])
                nc.vector.tensor_tensor(oh_g[:], iota_b, cm_b, OP.is_equal)
                oh_gs.append(oh_g)
            for g in range(NG):
                xno_g = gpool.tile([P, TG, HD], BF16, tag="xno_g", bufs=NG,
                                   name=f"xno{g}")
                xno_gs.append(xno_g)
                s_bd = s_gs[g][:].to_broadcast([P, TG, D])
                nc.gpsimd.tensor_tensor(xno_g[:, :, 0:D], b_gs[g][:],
                                        s_bd, OP.mult)
                nc.scalar.mul(xno_g[:, :, D:HD], oh_gs[g][:], sqk)
            # re-warm PE just before its dense block
            for i in range(8):
                wp = ps_w.tile([P, P], F32, tag="warm", name="wp")
                nc.tensor.matmul(wp[:], ident[:], ident[:],
                                 start=True, stop=True)
            for g in range(NG):
                # transposes first so hT groups complete ASAP (main-loop
                # dependency); only tiles the main loop actually reads
                for h in range(TG // 8):
                    tiles = [t for t in range(g * TG + 8 * h,
                                              g * TG + 8 * h + 8)
                             if t < t_used]
                    if not tiles:
                        continue
                    hps = ps_a.tile([HD, 8 * P], F32, tag="hps")
                    for q, t in enumerate(tiles):
                        nc.tensor.matmul(hps[:, q * P:(q + 1) * P],
                                         xno_gs[g][:, t - g * TG, :],
                                         ident[:], start=True, stop=True)
                    base = (g * TG + h * 8) * P - g * GB
                    cp = (nc.scalar.copy if (g + h) % 2 == 0
                          else nc.vector.tensor_copy)
                    cp(hT[g][:, base:base + len(tiles) * P],
                       hps[:, 0:len(tiles) * P])
                # S_ext = (sqk*O)^T @ [xn | sqk*O]: cols 0:D = sqk*S,
                # cols D:HD = sqk^2 * O^T O (diag = counts)
                for i in range(TG):
                    nc.tensor.matmul(s_ps[:], xno_gs[g][:, i, D:HD],
                                     xno_gs[g][:, i, :],
                                     start=(g == 0 and i == 0),
                                     stop=(g == NG - 1 and i == TG - 1))

        # ---- corr from S_ext (frees the S psum bank before main) ----
        # corr = (0.9-K)*A - 2*B with B = sum(s_ps[:,0:D]^2)/K and
        # A = sum(s_ps[:,D:HD]^2)/K^2
        sqs = persist.tile([C, D], F32)
        b_vec = persist.tile([C, 1], F32)
        nc.scalar.activation(sqs[:], s_ps[:, 0:D], AF.Square,
                             accum_out=b_vec[:])
        sqa = persist.tile([C, C], F32)
        a_vec = persist.tile([C, 1], F32)
        nc.scalar.activation(sqa[:], s_ps[:, D:HD], AF.Square,
                             accum_out=a_vec[:])
        t1 = persist.tile([C, 1], F32)
        nc.vector.tensor_scalar(t1[:], b_vec[:], -2.0 / K, None, OP.mult)
        corr_v = persist.tile([C, 1], F32)
        nc.vector.scalar_tensor_tensor(
            corr_v[:], a_vec[:], ((1.0 - m1) - K) / (K * K),
            t1[:], OP.mult, OP.add)
        prep_es.close()  # free prep pools (incl. S psum bank) before main

        # ---- main loop ----
        acc = {}
        acc[("A", 1)] = persist.tile([P, max(n_a[1], 1)], F32, name="accA1")
        acc[("A", 2)] = persist.tile([P, max(n_a[2], 1)], F32, name="accA2")
        acc[("D", 1)] = persist.tile([P, max(n_d[1], 1)], F32, name="accD1")
        acc[("D", 2)] = persist.tile([P, max(n_d[2], 1)], F32, name="accD2")
        nxt = {k: 0 for k in acc}
        with tc.tile_pool(name="ps_g", bufs=4, space="PSUM") as ps_g:
            for (segs, w, e) in sched:
                fd = sum(s[2] for s in segs)
                gt = ps_g.tile([P, fd], F32, name="gt", tag="g")
                x = 0
                for (r, off, width) in segs:
                    lhsT = hT[r // TG][:, (r % TG) * P:((r % TG) + 1) * P]
                    while width > 0:
                        mw = min(512 - (x % 512), width,
                                 GB - (off % GB))
                        nc.tensor.matmul(
                            gt[:, x:x + mw], lhsT,
                            hT[off // GB][:, off % GB:off % GB + mw],
                            start=True, stop=True)
                        x += mw
                        off += mw
                        width -= mw
                at = acc[(e, w)]
                i = nxt[(e, w)]
                nxt[(e, w)] += 1
                if e == "A":
                    nc.scalar.activation(gt[:], gt[:], AF.Relu,
                                         bias=bias_m1[:, 0:1], scale=1.0,
                                         accum_out=at[:, i:i + 1])
                else:
                    nc.vector.tensor_scalar(gt[:], gt[:], -m1, 0.0,
                                            OP.max, OP.add,
                                            accum_out=at[:, i:i + 1])

            # ---- final weighted reduction: red = r1 + 2*r2 ----
            reds = {}
            for (key, tl) in acc.items():
                rr = persist.tile([P, 1], F32, name=f"red{key[0]}{key[1]}")
                if nxt[key] == 0:
                    nc.gpsimd.memset(rr[:], 0.0)
                else:
                    nc.vector.tensor_reduce(rr[:], tl[:],
                                            axis=mybir.AxisListType.X,
                                            op=OP.add)
                reds[key] = rr
            r1 = persist.tile([P, 1], F32)
            nc.vector.tensor_add(r1[:], reds[("A", 1)][:], reds[("D", 1)][:])
            r2 = persist.tile([P, 1], F32)
            nc.vector.tensor_add(r2[:], reds[("A", 2)][:], reds[("D", 2)][:])
            red = persist.tile([P, 1], F32)
            nc.vector.scalar_tensor_tensor(red[:], r2[:], 2.0, r1[:],
                                           OP.mult, OP.add)
        with tc.tile_pool(name="ps_f", bufs=1, space="PSUM") as ps_f:
            corr_ps = ps_f.tile([1, 1], F32, tag="corr")
            nc.tensor.matmul(corr_ps[:], corr_v[:], ones_c[:],
                             start=True, stop=True)
            tot_ps = ps_f.tile([1, 1], F32, tag="tot")
            nc.tensor.matmul(tot_ps[:], red[:], ones128[:],
                             start=True, stop=True)
            f1 = persist.tile([1, 1], F32)
            nc.vector.tensor_scalar(f1[:], corr_ps[:], 1.0 / n_cores,
                                    None, OP.mult)
            f2 = persist.tile([1, 1], F32)
            nc.vector.scalar_tensor_tensor(f2[:], tot_ps[:], dve_off,
                                           f1[:], OP.add, OP.add)
            fin = persist.tile([1, 1], F32)
            nc.vector.tensor_scalar(fin[:], f2[:],
                                    1.0 / (float(N) * N), None, OP.mult)
            nc.sync.dma_start(out_dram[:], fin[:])

    nc.compile()
    return nc, dict(N=N, T=T, n_cores=n_cores, sched=sched)


def host_inputs(bottleneck, class_map, n_cores=8):
    """Full inputs -> per-core in_maps (rolled + tiled layouts)."""
    N, D = bottleneck.shape
    P = 128
    T = N // P
    roll = N // n_cores
    maps = []
    for c in range(n_cores):
        b = np.roll(bottleneck, -roll * c, axis=0)
        cm = np.roll(class_map, -roll * c, axis=0)
        b_t = np.ascontiguousarray(
            b.reshape(T, P, D).transpose(1, 0, 2))          # [128, T, D]
        cm_t = np.ascontiguousarray(cm.reshape(T, P).T)     # [128, T]
        maps.append({"b_t": b_t.astype(np.float32),
                     "cm_t": cm_t.astype(np.int32)})
    return maps


# ---------------------------------------------------------------------------
# Harness entry point: kernel(**inputs) takes the FULL unsharded inputs and
# returns the full (scalar) output. Shards by row-rolling across 8 cores,
# runs the SPMD Bass kernel, and sums the per-core partials on the host.
# ---------------------------------------------------------------------------
from concourse.bass_utils import run_bass_kernel_spmd

_CACHED = {}


def _get_nc():
    if "nc" not in _CACHED:
        _CACHED["nc"] = build_nc(N=8192, D=64, C=17, margin=1.1, n_cores=8,
                                 cw=1024, triangle=True)[0]
    return _CACHED["nc"]


def kernel(bottleneck, class_map):
    bottleneck = np.asarray(bottleneck, dtype=np.float32)
    class_map = np.asarray(class_map, dtype=np.int32)
    nc = _get_nc()
    maps = host_inputs(bottleneck, class_map, n_cores=8)
    res = run_bass_kernel_spmd(nc, maps, core_ids=list(range(8)))
    total = sum(float(r["out"][0, 0]) for r in res.results)
    return np.float32(total)



# revision 4
# speedup vs baseline: 1.0108x; 1.0108x over previous
"""Bass kernel for ClassSeparationLossMargin (v2).

loss = mean_ij [ t*(1-cos) + (1-t)*relu(margin - (1-cos)) ],
cos = xn @ xn.T (row-normalized), t = same-class mask, margin = 1.1.

Math (K-shift with K=1): let H = [xn | O] (O = one-hot classes), so
G = H H^T = cos + t.  For same-class pairs relu(0.1 + cos + 1) is exactly
linear (0.1 + cos + 1 >= 0.098), hence

  loss*N^2 = sum_pairs relu(G + 0.1) - 0.1*A - 2*B,
  A = sum_c n_c^2,  B = sum_c ||sum_{i in c} xn_i||^2.

The host prepares the "all-gathered normalized target copy" per the
sharding strategy: hT_c = H[(1024c + j) mod N].T as [81, 5120] bf16 for
core c (rows 0:64 normalized features, 64:81 one-hot).  Each core owns 8
row-tiles of the pair matrix and processes col chunks at tile distance
d in [0, 32]: weight 1 at d=0 and d=32 (d=32 pairs are covered by both
endpoint cores), weight 2 for d in [1, 31].  Summed over the 8 rolled
copies every ordered pair is counted exactly once.  A and B (plus the
final sum of 8 partials) are the "all-reduce of partial loss sums",
done on the host in fp64.

Device per core: DMA hT in 5 chunks of 1024 cols; stream of 34 Gram ops
through a 4-slot x 1024-col PSUM rotation; consumers split between
ACT (activation Relu bias=0.1, accum_out) and DVE
(scalar_tensor_tensor: acc = max(g, -0.1) + acc, i.e. relu(g+0.1)-0.1
per element, corrected on the host); tail reduces to [1,3] =
(w1_sum, w2_act_sum, w2_dve_sum).
"""

from contextlib import ExitStack

import numpy as np
import ml_dtypes

import concourse.bacc as bacc
import concourse.mybir as mybir
import concourse.tile as tile
from concourse.masks import make_identity

F32 = mybir.dt.float32
BF16 = mybir.dt.bfloat16
OP = mybir.AluOpType
AF = mybir.ActivationFunctionType

P = 128
N = 8192
D = 64
C = 17
HD = D + C            # 81
T = N // P            # 64 row tiles
RC = 8                # row chunks per core
NT = RC + 32          # hT col tiles used: 40
COLS = NT * P         # 5120
CHUNK = 1024
NCH = COLS // CHUNK   # 5
M1 = 0.1              # margin - 1


def _op_schedule():
    """Static consumer-op schedule.

    Returns list of ops: (name, fd, weight, segments, engine) where
    segments = [(col_lo, width, lhsT_r)] with lhsT_r the row chunk whose
    hT slice is the stationary operand; col offsets absolute into hT.
    Engine: 'A' (scalar/ACT) or 'D' (vector/DVE); weight-1 ops forced to
    ACT so the DVE running accumulator stays uniformly weight-2.
    """
    ops = []
    # d0: diagonal chunks (r, r), w1 -- earliest data (chunk 0 only)
    ops.append(("d0", 1024, 1, [(r * P, P, r) for r in range(RC)], "A"))
    # per-r spans at increasing d so ops unlock as DMA chunks land
    for dlo, dhi in ((1, 8), (9, 16), (17, 24), (25, 31)):
        w = (dhi - dlo + 1) * P
        for r in range(RC):
            ops.append((f"r{r}d{dlo}", w, 2, [((r + dlo) * P, w, r)], None))
    # d32: antipode chunks (r, r+32), w1 -- needs the last chunk
    ops.append(("d32", 1024, 1, [((r + 32) * P, P, r) for r in range(RC)], "A"))

    # engine assignment: greedy balance on modeled cost; w1 -> ACT.
    ca = lambda fd: (172 + fd) / 1.2 + 317   # ACTIVATE + READ_ACCUMULATOR
    cd = lambda fd: (120 + fd) / 0.96        # STT, no drain
    ta, td = 0.0, 1192.0                     # DVE reserves its tail reduce
    out = []
    for (name, fd, w, segs, e) in ops:
        if e is None:
            e = "A" if ta + ca(fd) <= td + cd(fd) else "D"
        if e == "A":
            ta += ca(fd)
        else:
            td += cd(fd)
        out.append((name, fd, w, segs, e))
    return out


def build_nc(n_cores=8):
    sched = _op_schedule()
    n_act = sum(1 for o in sched if o[4] == "A")
    dve_cols = sum(o[1] for o in sched if o[4] == "D")

    nc = bacc.Bacc("TRN2", target_bir_lowering=False, num_devices=n_cores)
    hT_dram = nc.dram_tensor("hT", [HD, COLS], BF16, kind="ExternalInput")
    out_dram = nc.dram_tensor("out", [1, 3], F32, kind="ExternalOutput")

    with tile.TileContext(nc) as tc, ExitStack() as top:
        persist = top.enter_context(tc.tile_pool(name="persist", bufs=1))

        ident = persist.tile([P, P], BF16)
        make_identity(nc, ident[:])
        bias_m1 = persist.tile([P, 1], F32)
        nc.gpsimd.memset(bias_m1[:], M1)
        ones128 = persist.tile([P, 1], F32)
        nc.gpsimd.memset(ones128[:], 1.0)
        accbuf = persist.tile([P, 1024], F32)
        nc.gpsimd.memset(accbuf[:], 0.0)
        acc_a = persist.tile([P, max(n_act, 1)], F32)
        stack3 = persist.tile([P, 3], F32)

        hT = [persist.tile([HD, CHUNK], BF16, name=f"hT{k}")
              for k in range(NCH)]
        for k in range(NCH):
            nc.sync.dma_start(hT[k][:], hT_dram[:, k * CHUNK:(k + 1) * CHUNK])

        def lhsT_of(r):
            return hT[0][:, r * P:(r + 1) * P]

        with tc.tile_pool(name="ps_g", bufs=1, space="PSUM") as ps_g:
            # HAM warm-up while the DMAs land (throwaway results in slot 3,
            # which the main stream only reaches at op 3)
            wt = ps_g.tile([P, 1024], F32, tag="g3", name="warm")
            for i in range(8):
                nc.tensor.matmul(wt[:, 0:P], ident[:], ident[:],
                                 start=True, stop=True)

            ia = 0
            for oi, (name, fd, w, segs, e) in enumerate(sched):
                gt = ps_g.tile([P, fd], F32, tag=f"g{oi % 4}", name=name)
                x = 0
                for (lo, width, r) in segs:
                    off = lo
                    while width > 0:
                        mw = min(512 - (x % 512), width,
                                 CHUNK - (off % CHUNK))
                        nc.tensor.matmul(
                            gt[:, x:x + mw], lhsT_of(r),
                            hT[off // CHUNK][:, off % CHUNK:off % CHUNK + mw],
                            start=True, stop=True)
                        x += mw
                        off += mw
                        width -= mw
                if e == "A":
                    nc.scalar.activation(gt[:], gt[:], AF.Relu,
                                         bias=bias_m1[:, 0:1], scale=1.0,
                                         accum_out=acc_a[:, ia:ia + 1])
                    ia += 1
                else:
                    nc.vector.scalar_tensor_tensor(
                        accbuf[:, 0:fd], gt[:], -M1, accbuf[:, 0:fd],
                        OP.max, OP.add)

            # ---- tail: 3 partition-wise sums -> ones-dot -> [1,3] ----
            w1_cols = [i for i, o in enumerate(
                [o for o in sched if o[4] == "A"]) if o[2] == 1]
            w2_cols = [i for i, o in enumerate(
                [o for o in sched if o[4] == "A"]) if o[2] == 2]
            # ACT accum column order == emission order; w1 ops are first
            # (d0) and last (d32) -> reduce the two slices separately.
            assert w1_cols == [0, len(w1_cols + w2_cols) - 1], w1_cols
            nc.vector.tensor_reduce(stack3[:, 1:2], acc_a[:, 1:n_act - 1],
                                    axis=mybir.AxisListType.X, op=OP.add)
            nc.vector.tensor_add(stack3[:, 0:1], acc_a[:, 0:1],
                                 acc_a[:, n_act - 1:n_act])
            nc.vector.tensor_reduce(stack3[:, 2:3], accbuf[:],
                                    axis=mybir.AxisListType.X, op=OP.add)
            fin_ps = ps_g.tile([1, 3], F32, tag="g0", name="fin")
            nc.tensor.matmul(fin_ps[:], ones128[:], stack3[:],
                             start=True, stop=True)
            fin = persist.tile([1, 3], F32)
            nc.vector.tensor_copy(fin[:], fin_ps[:])
            nc.sync.dma_start(out_dram[:], fin[:])

    nc.compile()
    return nc, dict(dve_cols=dve_cols)


# ---------------------------------------------------------------------------
# Host side
# ---------------------------------------------------------------------------

def host_maps(bottleneck, class_map, n_cores=8):
    b = np.asarray(bottleneck, dtype=np.float32)
    cm = np.asarray(class_map, dtype=np.int64)
    norm = np.sqrt((b.astype(np.float64) ** 2).sum(axis=1, keepdims=True))
    xn = (b / np.maximum(norm, 1e-8)).astype(np.float32)
    oh = (cm[:, None] == np.arange(C)[None, :]).astype(np.float32)
    X = np.concatenate([xn, oh], axis=1).astype(ml_dtypes.bfloat16)  # [N, 81]
    roll = N // n_cores
    maps = []
    for c in range(n_cores):
        idx = (roll * c + np.arange(COLS)) % N
        maps.append({"hT": np.ascontiguousarray(X[idx].T)})      # [81, 5120]
    # closed-form same-class correction (fp64)
    counts = np.bincount(cm, minlength=C).astype(np.float64)
    A = float((counts ** 2).sum())
    S = oh.astype(np.float64).T @ xn.astype(np.float64)          # [C, D]
    B = float((S ** 2).sum())
    return maps, A, B


def combine(results, A, B, dve_cols):
    total = 0.0
    for r in results:
        w1, w2a, w2d = (float(v) for v in np.asarray(r["out"]).ravel())
        w2d += M1 * P * dve_cols      # DVE path computes relu(g+0.1)-0.1
        total += w1 + 2.0 * (w2a + w2d)
    total += -M1 * A - 2.0 * B
    return np.float32(total / (float(N) * N))


from concourse.bass_utils import run_bass_kernel_spmd

_CACHED = {}


def _get_nc():
    if "nc" not in _CACHED:
        _CACHED["nc"] = build_nc(n_cores=8)
    return _CACHED["nc"]


def kernel(bottleneck, class_map):
    nc, meta = _get_nc()
    maps, A, B = host_maps(bottleneck, class_map, n_cores=8)
    res = run_bass_kernel_spmd(nc, maps, core_ids=list(range(8)))
    return combine(res.results, A, B, meta["dve_cols"])


# revision 5
# speedup vs baseline: 1.0475x; 1.0364x over previous
"""Bass kernel for ClassSeparationLossMargin (v3).

loss = mean_ij [ t*(1-cos) + (1-t)*relu(margin - (1-cos)) ],
cos = xn @ xn.T (row-normalized), t = same-class mask, margin = 1.1.

Math (K-shift with K=1): let H = [xn | O] (O = one-hot classes), so
G = H H^T = cos + t.  For same-class pairs relu(0.1 + cos + 1) is exactly
linear (0.1 + cos + 1 >= 0.098), hence

  loss*N^2 = sum_pairs relu(G + 0.1) - 0.1*A - 2*B,
  A = sum_c n_c^2,  B = sum_c ||sum_{i in c} xn_i||^2.

The host prepares the "all-gathered normalized target copy" per the
sharding strategy: hT_c = H[(1024c + j) mod N].T as [128, 5120] bf16 for
core c (rows 0:64 normalized features, 64:81 one-hot, 81:128 zero pad so
the 128-column stationary loads take the fast-weight-load path).  Each
core owns 8 row-tiles of the pair matrix and processes col chunks at
tile distance d in [0, 32]: weight 1 at d=0 and d=32 (d=32 pairs are
covered by both endpoint cores), weight 2 for d in [1, 31].  Summed over
the 8 rolled copies every ordered pair is counted exactly once.  A and B
(plus the final sum of the 8 partials) are the "all-reduce of partial
loss sums", done on the host in fp64.

Device per core: DMA hT in 2 chunks; Gram ops stream through a 3-slot
PSUM rotation (1536/1536/1024 f32 cols = all 8 banks); consumers split
between ACT (activation Relu bias=0.1, accum_out, out to bf16 SBUF
scratch) and DVE (tensor_scalar max(g,-0.1)+0, accum_out; undercounts
0.1/elem, corrected on host); tail reduces the accum columns to [1,3] =
(w1_sum, w2_act_sum, w2_dve_sum).
"""

from contextlib import ExitStack

import numpy as np
import ml_dtypes

import concourse.bacc as bacc
import concourse.mybir as mybir
import concourse.tile as tile
from concourse.masks import make_identity

F32 = mybir.dt.float32
BF16 = mybir.dt.bfloat16
OP = mybir.AluOpType
AF = mybir.ActivationFunctionType

P = 128
N = 8192
D = 64
C = 17
HD = D + C            # 81 real rows; padded to 128 in DRAM/SBUF
HP = 128              # padded stationary partition count
RC = 8                # row chunks per core
NT = RC + 32          # hT col tiles used: 40
COLS = NT * P         # 5120
CHUNK = 2560
NCH = COLS // CHUNK   # 2
M1 = 0.1              # margin - 1


def _op_schedule():
    """[(name, fd, weight, segs, engine, slot)]; segs=[(col_lo, width, r)].

    Slot rotation A(1536) B(1536) C(1024): per r the w2 span d in [1,31]
    splits as 1536+1536+896 (A,B,C); d0 / d32 chunk-batches (w1) use C.
    Weight-1 ops go to ACT (exact relu); DVE ops all weight-2.
    """
    ops = []
    ops.append(("d0", 1024, 1, [(r * P, P, r) for r in range(RC)], "A", "C"))
    for r in range(RC):
        ops.append((f"r{r}a", 1536, 2, [((r + 1) * P, 1536, r)], None, "A"))
        ops.append((f"r{r}b", 1536, 2, [((r + 13) * P, 1536, r)], None, "B"))
        ops.append((f"r{r}c", 896, 2, [((r + 25) * P, 896, r)], None, "C"))
    ops.append(("d32", 1024, 1,
                [((r + 32) * P, P, r) for r in range(RC)], "A", "C"))

    # greedy engine balance with measured-cost model (ns)
    ca = lambda fd: fd / 1.2 + 1030    # ACTIVATE + drain + sem overhead
    cd = lambda fd: fd / 0.96 + 660    # TENSOR_SCALAR + drain + sem
    ta = td = 0.0
    out = []
    for (name, fd, w, segs, e, sl) in ops:
        if e is None:
            e = "A" if ta + ca(fd) <= td + cd(fd) else "D"
        if e == "A":
            ta += ca(fd)
        else:
            td += cd(fd)
        out.append((name, fd, w, segs, e, sl))
    return out


def build_nc(n_cores=8):
    sched = _op_schedule()
    n_act = sum(1 for o in sched if o[4] == "A")
    n_dve = sum(1 for o in sched if o[4] == "D")
    dve_cols = sum(o[1] for o in sched if o[4] == "D")

    nc = bacc.Bacc("TRN2", target_bir_lowering=False, num_devices=n_cores)
    hT_dram = nc.dram_tensor("hT", [HP, COLS], BF16, kind="ExternalInput")
    out_dram = nc.dram_tensor("out", [1, 3], F32, kind="ExternalOutput")

    slot_fd = {"A": 1536, "B": 1536, "C": 1024}

    with tile.TileContext(nc) as tc, ExitStack() as top:
        persist = top.enter_context(tc.tile_pool(name="persist", bufs=1))

        ident = persist.tile([P, P], BF16)
        make_identity(nc, ident[:])
        bias_m1 = persist.tile([P, 1], F32)
        nc.gpsimd.memset(bias_m1[:], M1)
        ones128 = persist.tile([P, 1], F32)
        nc.gpsimd.memset(ones128[:], 1.0)
        scr_a = persist.tile([P, 1536], BF16)     # ACT relu output sink
        acc_a = persist.tile([P, max(n_act, 1)], F32)
        acc_d = persist.tile([P, max(n_dve, 1)], F32)
        stack3 = persist.tile([P, 3], F32)

        hT = [persist.tile([HP, CHUNK], BF16, name=f"hT{k}")
              for k in range(NCH)]
        for k in range(NCH):
            nc.sync.dma_start(hT[k][:], hT_dram[:, k * CHUNK:(k + 1) * CHUNK])

        def lhsT_of(r):
            return hT[0][:, r * P:(r + 1) * P]

        with tc.tile_pool(name="ps_g", bufs=1, space="PSUM") as ps_g:
            # HAM warm-up while the DMAs land (throwaway results into the
            # C slot, which the main stream reuses with a plain WAW dep)
            wt = ps_g.tile([P, 1024], F32, tag="C", name="warm")
            for i in range(12):
                nc.tensor.matmul(wt[:, 0:P], ident[:], ident[:],
                                 start=True, stop=True)

            ia = id_ = 0
            for (name, fd, w, segs, e, sl) in sched:
                gt = ps_g.tile([P, slot_fd[sl]], F32, tag=sl, name=name)
                x = 0
                for (lo, width, r) in segs:
                    off = lo
                    while width > 0:
                        mw = min(512 - (x % 512), width,
                                 CHUNK - (off % CHUNK))
                        nc.tensor.matmul(
                            gt[:, x:x + mw], lhsT_of(r),
                            hT[off // CHUNK][:, off % CHUNK:off % CHUNK + mw],
                            start=True, stop=True)
                        x += mw
                        off += mw
                        width -= mw
                if e == "A":
                    nc.scalar.activation(scr_a[:, 0:fd], gt[:, 0:fd], AF.Relu,
                                         bias=bias_m1[:, 0:1], scale=1.0,
                                         accum_out=acc_a[:, ia:ia + 1])
                    ia += 1
                else:
                    nc.vector.tensor_scalar(gt[:, 0:fd], gt[:, 0:fd],
                                            -M1, 0.0, OP.max, OP.add,
                                            accum_out=acc_d[:, id_:id_ + 1])
                    id_ += 1

            # ---- tail: 3 partition sums -> ones-dot -> [1,3] ----
            a_ops = [o for o in sched if o[4] == "A"]
            w1_idx = [i for i, o in enumerate(a_ops) if o[2] == 1]
            assert w1_idx == [0, n_act - 1], w1_idx
            nc.vector.tensor_add(stack3[:, 0:1], acc_a[:, 0:1],
                                 acc_a[:, n_act - 1:n_act])
            nc.vector.tensor_reduce(stack3[:, 1:2], acc_a[:, 1:n_act - 1],
                                    axis=mybir.AxisListType.X, op=OP.add)
            nc.vector.tensor_reduce(stack3[:, 2:3], acc_d[:, 0:n_dve],
                                    axis=mybir.AxisListType.X, op=OP.add)
            fin_ps = ps_g.tile([1, 3], F32, tag="C", name="fin")
            nc.tensor.matmul(fin_ps[:], ones128[:], stack3[:],
                             start=True, stop=True)
            fin = persist.tile([1, 3], F32)
            nc.vector.tensor_copy(fin[:], fin_ps[:])
            nc.sync.dma_start(out_dram[:], fin[:])

    nc.compile()
    return nc, dict(dve_cols=dve_cols)


# ---------------------------------------------------------------------------
# Host side
# ---------------------------------------------------------------------------

def host_maps(bottleneck, class_map, n_cores=8):
    b = np.asarray(bottleneck, dtype=np.float32)
    cm = np.asarray(class_map, dtype=np.int64)
    norm = np.sqrt((b.astype(np.float64) ** 2).sum(axis=1, keepdims=True))
    xn = (b / np.maximum(norm, 1e-8)).astype(np.float32)
    oh = (cm[:, None] == np.arange(C)[None, :]).astype(np.float32)
    X = np.zeros((N, HP), dtype=ml_dtypes.bfloat16)
    X[:, 0:D] = xn
    X[:, D:HD] = oh
    roll = N // n_cores
    maps = []
    for c in range(n_cores):
        idx = (roll * c + np.arange(COLS)) % N
        maps.append({"hT": np.ascontiguousarray(X[idx].T)})      # [128, 5120]
    counts = np.bincount(cm, minlength=C).astype(np.float64)
    A = float((counts ** 2).sum())
    S = oh.astype(np.float64).T @ xn.astype(np.float64)          # [C, D]
    B = float((S ** 2).sum())
    return maps, A, B


def combine(results, A, B, dve_cols):
    total = 0.0
    for r in results:
        w1, w2a, w2d = (float(v) for v in np.asarray(r["out"]).ravel())
        w2d += M1 * P * dve_cols      # DVE path computes relu(g+0.1)-0.1
        total += w1 + 2.0 * (w2a + w2d)
    total += -M1 * A - 2.0 * B
    return np.float32(total / (float(N) * N))


from concourse.bass_utils import run_bass_kernel_spmd

_CACHED = {}


def _get_nc():
    if "nc" not in _CACHED:
        _CACHED["nc"] = build_nc(n_cores=8)
    return _CACHED["nc"]


def kernel(bottleneck, class_map):
    nc, meta = _get_nc()
    maps, A, B = host_maps(bottleneck, class_map, n_cores=8)
    res = run_bass_kernel_spmd(nc, maps, core_ids=list(range(8)))
    return combine(res.results, A, B, meta["dve_cols"])


# revision 7
# speedup vs baseline: 1.2736x; 1.2157x over previous
"""Bass kernel for ClassSeparationLossMargin (v3).

loss = mean_ij [ t*(1-cos) + (1-t)*relu(margin - (1-cos)) ],
cos = xn @ xn.T (row-normalized), t = same-class mask, margin = 1.1.

Math (K-shift with K=1): let H = [xn | O] (O = one-hot classes), so
G = H H^T = cos + t.  For same-class pairs relu(0.1 + cos + 1) is exactly
linear (0.1 + cos + 1 >= 0.098), hence

  loss*N^2 = sum_pairs relu(G + 0.1) - 0.1*A - 2*B,
  A = sum_c n_c^2,  B = sum_c ||sum_{i in c} xn_i||^2.

The host prepares the "all-gathered normalized target copy" per the
sharding strategy: hT_c = H[(1024c + j) mod N].T as [128, 5120] bf16 for
core c (rows 0:64 normalized features, 64:81 one-hot, 81:128 zero pad so
the 128-column stationary loads take the fast-weight-load path).  Each
core owns 8 row-tiles of the pair matrix and processes col chunks at
tile distance d in [0, 32]: weight 1 at d=0 and d=32 (d=32 pairs are
covered by both endpoint cores), weight 2 for d in [1, 31].  Summed over
the 8 rolled copies every ordered pair is counted exactly once.  A and B
(plus the final sum of the 8 partials) are the "all-reduce of partial
loss sums", done on the host in fp64.

Device per core: DMA hT in 2 chunks; Gram ops stream through a 3-slot
PSUM rotation (1536/1536/1024 f32 cols = all 8 banks); consumers split
between ACT (activation Relu bias=0.1, accum_out, out to bf16 SBUF
scratch) and DVE (tensor_scalar max(g,-0.1)+0, accum_out; undercounts
0.1/elem, corrected on host); tail reduces the accum columns to [1,3] =
(w1_sum, w2_act_sum, w2_dve_sum).
"""

from contextlib import ExitStack

import numpy as np
import ml_dtypes

import concourse.bacc as bacc
import concourse.mybir as mybir
import concourse.tile as tile
from concourse.masks import make_identity

F32 = mybir.dt.float32
BF16 = mybir.dt.bfloat16
OP = mybir.AluOpType
AF = mybir.ActivationFunctionType

P = 128
N = 8192
D = 64
C = 17
HD = D + C            # 81 real rows; padded to 128 in DRAM/SBUF
HP = 128              # padded stationary partition count
RC = 8                # row chunks per core
NT = RC + 32          # hT col tiles used: 40
COLS = NT * P         # 5120
CHUNKS = (512, 1152, 1152, 1152, 1152)  # first chunk small: d0 unlocks early
M1 = 0.1              # margin - 1


def _op_schedule():
    """[(name, fd, weight, segs, engine, slot)]; segs=[(col_lo, width, r)].

    Slot rotation A(1536) B(1536) C(1024): per r the w2 span d in [1,31]
    splits as 1536+1536+896 (A,B,C); d0 / d32 chunk-batches (w1) use C.
    Weight-1 ops go to ACT (exact relu); DVE ops all weight-2.
    """
    ops = []
    ops.append(("d0", 1024, 1, [(r * P, P, r) for r in range(RC)], None, "C"))
    for r in range(RC):
        ops.append((f"r{r}a", 1536, 2, [((r + 1) * P, 1536, r)], None, "A"))
        ops.append((f"r{r}b", 1536, 2, [((r + 13) * P, 1536, r)], None, "B"))
        ops.append((f"r{r}c", 896, 2, [((r + 25) * P, 896, r)], None, "C"))
    ops.append(("d32", 1024, 1,
                [((r + 32) * P, P, r) for r in range(RC)], None, "C"))

    # greedy engine balance with measured-cost model (ns)
    ca = lambda fd: fd / 1.2 + 1080    # ACTIVATE + drain + sem overhead
    cd = lambda fd: fd / 0.96 + 780    # TENSOR_SCALAR + drain + sem
    ta = td = 0.0
    out = []
    for (name, fd, w, segs, e, sl) in ops:
        if e is None:
            e = "A" if ta + ca(fd) <= td + cd(fd) else "D"
        if e == "A":
            ta += ca(fd)
        else:
            td += cd(fd)
        out.append((name, fd, w, segs, e, sl))
    return out


def build_nc(n_cores=8):
    sched = _op_schedule()
    n_act = sum(1 for o in sched if o[4] == "A")
    n_dve = sum(1 for o in sched if o[4] == "D")
    dve_cols = sum(o[1] for o in sched if o[4] == "D")

    nc = bacc.Bacc("TRN2", target_bir_lowering=False, num_devices=n_cores)
    hT_dram = nc.dram_tensor("hT", [HP, COLS], BF16, kind="ExternalInput")
    out_dram = nc.dram_tensor("out", [P, n_act + n_dve], F32,
                              kind="ExternalOutput")

    slot_fd = {"A": 1536, "B": 1536, "C": 1024}

    with tile.TileContext(nc) as tc, ExitStack() as top:
        persist = top.enter_context(tc.tile_pool(name="persist", bufs=1))

        ident = persist.tile([P, P], BF16)
        make_identity(nc, ident[:])
        bias_m1 = persist.tile([P, 1], F32)
        nc.gpsimd.memset(bias_m1[:], M1)
        acc_a = persist.tile([P, max(n_act, 1)], F32)
        acc_d = persist.tile([P, max(n_dve, 1)], F32)

        bounds = [0]
        for w in CHUNKS:
            bounds.append(bounds[-1] + w)
        hT = [persist.tile([HP, CHUNKS[k]], BF16, name=f"hT{k}")
              for k in range(len(CHUNKS))]
        for k in range(len(CHUNKS)):
            nc.sync.dma_start(hT[k][:], hT_dram[:, bounds[k]:bounds[k + 1]])

        def chunk_of(col):
            for k in range(len(CHUNKS)):
                if col < bounds[k + 1]:
                    return k, col - bounds[k]
            raise AssertionError(col)

        def lhsT_of(r):
            k, o = chunk_of(r * P)
            assert o + P <= CHUNKS[k]
            return hT[k][:, o:o + P]

        with tc.tile_pool(name="ps_g", bufs=1, space="PSUM") as ps_g:
            # HAM warm-up while the DMAs land (throwaway results into the
            # C slot, which the main stream reuses with a plain WAW dep)
            wt = ps_g.tile([P, 1024], F32, tag="C", name="warm")
            for i in range(6):
                nc.tensor.matmul(wt[:, 0:P], ident[:], ident[:],
                                 start=True, stop=True)

            ia = id_ = 0
            for (name, fd, w, segs, e, sl) in sched:
                gt = ps_g.tile([P, slot_fd[sl]], F32, tag=sl, name=name)
                x = 0
                for (lo, width, r) in segs:
                    off = lo
                    while width > 0:
                        k, o = chunk_of(off)
                        mw = min(512 - (x % 512), width, CHUNKS[k] - o)
                        nc.tensor.matmul(
                            gt[:, x:x + mw], lhsT_of(r),
                            hT[k][:, o:o + mw],
                            start=True, stop=True)
                        x += mw
                        off += mw
                        width -= mw
                if e == "A":
                    nc.scalar.activation(gt[:, 0:fd], gt[:, 0:fd], AF.Relu,
                                         bias=bias_m1[:, 0:1], scale=1.0,
                                         accum_out=acc_a[:, ia:ia + 1])
                    ia += 1
                else:
                    nc.vector.tensor_scalar(gt[:, 0:fd], gt[:, 0:fd],
                                            -M1, 0.0, OP.max, OP.add,
                                            accum_out=acc_d[:, id_:id_ + 1])
                    id_ += 1

            # ---- tail: ship the raw accumulator columns; the host does
            # the final (tiny) weighted reduction in fp64 ----
            nc.sync.dma_start(out_dram[:, 0:n_act], acc_a[:, 0:n_act])
            nc.sync.dma_start(out_dram[:, n_act:n_act + n_dve],
                              acc_d[:, 0:n_dve])

    nc.compile()
    cols = ([(o[2], o[1], "A") for o in sched if o[4] == "A"] +
            [(o[2], o[1], "D") for o in sched if o[4] == "D"])
    return nc, dict(cols=cols)


# ---------------------------------------------------------------------------
# Host side
# ---------------------------------------------------------------------------

def host_maps(bottleneck, class_map, n_cores=8):
    b = np.asarray(bottleneck, dtype=np.float32)
    cm = np.asarray(class_map, dtype=np.int64)
    norm = np.sqrt((b.astype(np.float64) ** 2).sum(axis=1, keepdims=True))
    xn = (b / np.maximum(norm, 1e-8)).astype(np.float32)
    oh = (cm[:, None] == np.arange(C)[None, :]).astype(np.float32)
    X = np.zeros((N, HP), dtype=ml_dtypes.bfloat16)
    X[:, 0:D] = xn
    X[:, D:HD] = oh
    roll = N // n_cores
    maps = []
    for c in range(n_cores):
        idx = (roll * c + np.arange(COLS)) % N
        maps.append({"hT": np.ascontiguousarray(X[idx].T)})      # [128, 5120]
    counts = np.bincount(cm, minlength=C).astype(np.float64)
    A = float((counts ** 2).sum())
    S = oh.astype(np.float64).T @ xn.astype(np.float64)          # [C, D]
    B = float((S ** 2).sum())
    return maps, A, B


def combine(results, A, B, cols):
    wvec = np.array([w for (w, fd, e) in cols], dtype=np.float64)
    # DVE columns hold sum(max(g,-0.1)) = sum(relu(g+0.1)) - 0.1*P*fd
    off = np.array([M1 * P * fd if e == "D" else 0.0
                    for (w, fd, e) in cols], dtype=np.float64)
    total = 0.0
    for r in results:
        colsum = np.asarray(r["out"]).astype(np.float64).sum(axis=0)
        total += float(((colsum + off) * wvec).sum())
    total += -M1 * A - 2.0 * B
    return np.float32(total / (float(N) * N))


from concourse.bass_utils import run_bass_kernel_spmd

_CACHED = {}


def _get_nc():
    if "nc" not in _CACHED:
        _CACHED["nc"] = build_nc(n_cores=8)
    return _CACHED["nc"]


def kernel(bottleneck, class_map):
    nc, meta = _get_nc()
    maps, A, B = host_maps(bottleneck, class_map, n_cores=8)
    res = run_bass_kernel_spmd(nc, maps, core_ids=list(range(8)))
    return combine(res.results, A, B, meta["cols"])
